# revision 1
# baseline (speedup 1.0000x reference)
"""Trainium2 Bass kernel for nn_CropperQAT (multi-scale RoIAlign with
fake-quantized rois).

Strategy (pure data-parallel over (roi, scale) jobs, 8 cores):
  * Host: replicate the reference's roi/coordinate math exactly in numpy
    (bit-exact), build per-(roi,scale) "jobs": per output row i the two
    source feature rows (bilinear y-neighbors) as flat pixel indices into a
    channels-last concatenated feature tensor, plus interpolation weights.
  * Device per job, partition = (job, i) [16 jobs x 8 rows = 128 partitions]:
      - 2 indirect DMA gathers: 9-pixel rows (576 f32) for the y-lo and y-hi
        neighbors of each output row.
      - y-interp: U = g_lo*wl + g_hi*wh   (ACT copy-scale + DVE stt; weights
        are per-partition scalars, y-validity mask folded in)
      - x-interp:
          x-regular jobs (consecutive xl, xh=xl+1, const fx, all valid):
             O = U[0:8]*bx0 + U[1:9]*bx1  (shifted slices, per-part scalars)
          x-irregular jobs: dense 9-term O[j] = sum_e U[e]*M[e,j]
      - contiguous DMA of O to per-job output slots; host re-permutes.
"""
import os
import sys

sys.path.insert(0, "/opt/trn_rl_repo")

import numpy as np

import concourse.bass as bass
import concourse.bacc as bacc
import concourse.mybir as mybir
from concourse.tile import TileContext
from concourse.bass_utils import run_bass_kernel_spmd

F32 = np.float32
SIZE = 8
STRIDES = (4, 8, 16)
QS = np.float32(0.25)
C = 64
N_CORES = 8
JOBS_PER_GROUP = 16          # partitions = 16 jobs x 8 rows
P = 128
IRRW = 76                    # irr weight row: wl, wh, pad, pad, M[9*8]

LAST_RESULTS = None          # BassKernelResults of the most recent run


# ----------------------------------------------------------------------------
# host-side math (bit-exact replication of the jax reference)
# ----------------------------------------------------------------------------

def _fake_quant(x):
    return (np.clip(np.round(x / QS), -32768, 32767) * QS).astype(F32)


def _prep(c, L):
    """Vectorized replica of reference.prep on [A, S] f32 coords."""
    valid = (c >= -1.0) & (c <= L)
    c = np.maximum(c, F32(0.0))
    low0 = np.floor(c).astype(np.int32)
    hi_edge = low0 >= L - 1
    low = np.where(hi_edge, L - 1, low0).astype(np.int32)
    high = np.where(hi_edge, L - 1, low0 + 1).astype(np.int32)
    c = np.where(hi_edge, F32(L - 1), c).astype(F32)
    frac = (c - low.astype(F32)).astype(F32)
    return low, high, frac, valid


def _scale_tables(pixel, batch_index, stride, H, W, base):
    A = pixel.shape[0]
    st = F32(stride)
    half = F32(SIZE / 2.0)
    centers = (np.arange(SIZE, dtype=F32) + F32(0.5)).astype(F32)

    px = pixel[:, 0].astype(F32)
    py = pixel[:, 1].astype(F32)
    x1 = _fake_quant(np.maximum(px / st - half, F32(0.0)).astype(F32))
    y1 = _fake_quant(np.maximum(py / st - half, F32(0.0)).astype(F32))
    x2 = _fake_quant(np.maximum(px / st + half, F32(0.0)).astype(F32))
    y2 = _fake_quant(np.maximum(py / st + half, F32(0.0)).astype(F32))
    roi_w = np.maximum(x2 - x1, F32(1.0)).astype(F32)
    roi_h = np.maximum(y2 - y1, F32(1.0)).astype(F32)
    y = (y1[:, None] + centers[None, :] * (roi_h / F32(SIZE))[:, None]).astype(F32)
    x = (x1[:, None] + centers[None, :] * (roi_w / F32(SIZE))[:, None]).astype(F32)

    yl, yh, fy, vy = _prep(y, H)
    xl, xh, fx, vx = _prep(x, W)

    b = batch_index.astype(np.int64)
    x0 = np.minimum(xl[:, 0], W - 9).astype(np.int64)   # 9-px window start
    row_lo = base + (b[:, None] * H + yl.astype(np.int64)) * W + x0[:, None]
    row_hi = base + (b[:, None] * H + yh.astype(np.int64)) * W + x0[:, None]

    vyf = vy.astype(F32)
    wl = ((F32(1.0) - fy) * vyf).astype(F32)            # [A, 8] per (job, i)
    wh = (fy * vyf).astype(F32)

    # x-regular classification
    ar = np.arange(SIZE, dtype=np.int32)
    reg = (np.all(xl == xl[:, :1] + ar[None, :], axis=1)
           & np.all(xh == xl + 1, axis=1)
           & np.all(vx, axis=1)
           & np.all(fx == fx[:, :1], axis=1))

    bx0 = (F32(1.0) - fx[:, 0]).astype(F32)
    bx1 = fx[:, 0].astype(F32)

    # dense x matrix for irregular jobs: M[a, e, j]
    e_lo = (xl.astype(np.int64) - x0[:, None])
    e_hi = (xh.astype(np.int64) - x0[:, None])
    assert e_lo.min() >= 0 and e_lo.max() <= 8 and e_hi.min() >= 0 and e_hi.max() <= 8
    vxf = vx.astype(F32)
    M = np.zeros((A, 9, SIZE), F32)
    aa = np.repeat(np.arange(A), SIZE)
    jj = np.tile(np.arange(SIZE), A)
    np.add.at(M, (aa, e_lo.ravel(), jj), ((F32(1.0) - fx) * vxf).ravel())
    np.add.at(M, (aa, e_hi.ravel(), jj), (fx * vxf).ravel())

    return dict(row_lo=row_lo, row_hi=row_hi, wl=wl, wh=wh,
                bx0=bx0, bx1=bx1, M=M, reg=reg)


def _host_prep(f0, f1, f2, pixel, batch_index):
    A = pixel.shape[0]
    feats = (f0, f1, f2)
    shapes = [(f.shape[2], f.shape[3]) for f in feats]

    cat = np.concatenate([
        np.ascontiguousarray(np.asarray(f, dtype=F32).transpose(0, 2, 3, 1)).reshape(-1, C)
        for f in feats], axis=0)

    tabs = []
    base = 0
    for s, (H, W) in enumerate(shapes):
        tabs.append(_scale_tables(np.asarray(pixel, F32), np.asarray(batch_index),
                                  STRIDES[s], H, W, base))
        base += 4 * H * W

    # ---- job lists (scale, a) split by x-regularity, round-robin over cores
    reg_jobs = np.array([(s, a) for s in range(3)
                         for a in np.nonzero(tabs[s]["reg"])[0]], dtype=np.int64)
    irr_jobs = np.array([(s, a) for s in range(3)
                         for a in np.nonzero(~tabs[s]["reg"])[0]], dtype=np.int64)
    if len(irr_jobs) == 0:
        irr_jobs = reg_jobs[:1].copy()
    core_reg = [reg_jobs[c::N_CORES] for c in range(N_CORES)]
    core_irr = [irr_jobs[c::N_CORES] for c in range(N_CORES)]
    NRG = max(-(-max(len(x) for x in core_reg) // JOBS_PER_GROUP), 1)
    NIG = max(-(-max(len(x) for x in core_irr) // JOBS_PER_GROUP), 1)
    NSLOT = (NRG + NIG) * JOBS_PER_GROUP

    per_core = []
    for ccc in range(N_CORES):
        entries = []   # (kind, jobs padded)
        rj = core_reg[ccc]
        ij = core_irr[ccc]
        if len(rj) == 0:
            rj = reg_jobs[:1].copy()
        if len(ij) == 0:
            ij = irr_jobs[:1].copy()
        rpad = np.concatenate([rj, np.repeat(rj[-1:], NRG * JOBS_PER_GROUP - len(rj),
                                             axis=0)])
        ipad = np.concatenate([ij, np.repeat(ij[-1:], NIG * JOBS_PER_GROUP - len(ij),
                                             axis=0)])

        def job_rows(jobs):
            # per (job, i) arrays, [len*8]
            s_arr, a_arr = jobs[:, 0], jobs[:, 1]
            rl = np.stack([tabs[s]["row_lo"][a] for s, a in jobs]).astype(np.int32)
            rh = np.stack([tabs[s]["row_hi"][a] for s, a in jobs]).astype(np.int32)
            wl = np.stack([tabs[s]["wl"][a] for s, a in jobs]).astype(F32)
            wh = np.stack([tabs[s]["wh"][a] for s, a in jobs]).astype(F32)
            return rl, rh, wl, wh, s_arr, a_arr

        rl, rh, wl, wh, _, _ = job_rows(rpad)
        reg_idx = np.stack([rl.reshape(-1), rh.reshape(-1)], axis=1)  # [NRG*128, 2]
        bx0 = np.stack([tabs[s]["bx0"][a] for s, a in rpad]).astype(F32)
        bx1 = np.stack([tabs[s]["bx1"][a] for s, a in rpad]).astype(F32)
        reg_w = np.stack([wl.reshape(-1), wh.reshape(-1),
                          np.repeat(bx0, SIZE), np.repeat(bx1, SIZE)], axis=1)

        rl, rh, wl, wh, _, _ = job_rows(ipad)
        irr_idx = np.stack([rl.reshape(-1), rh.reshape(-1)], axis=1)
        Mm = np.stack([tabs[s]["M"][a] for s, a in ipad]).astype(F32)  # [NI*16? , 9, 8]
        irr_w = np.zeros((NIG * P, IRRW), F32)
        irr_w[:, 0] = wl.reshape(-1)
        irr_w[:, 1] = wh.reshape(-1)
        irr_w[:, 4:] = np.repeat(Mm.reshape(len(ipad), 72), SIZE, axis=0)

        # partition-major packing: [128, NG*k]
        reg_idx_pm = reg_idx.reshape(NRG, P, 2).transpose(1, 0, 2).reshape(P, NRG * 2)
        reg_w_pm = reg_w.reshape(NRG, P, 4).transpose(1, 0, 2).reshape(P, NRG * 4)
        irr_idx_pm = irr_idx.reshape(NIG, P, 2).transpose(1, 0, 2).reshape(P, NIG * 2)
        irr_w_pm = irr_w.reshape(NIG, P, IRRW).transpose(1, 0, 2).reshape(P, NIG * IRRW)
        per_core.append(dict(reg_idx=np.ascontiguousarray(reg_idx_pm, np.int32),
                             reg_w=np.ascontiguousarray(reg_w_pm, F32),
                             irr_idx=np.ascontiguousarray(irr_idx_pm, np.int32),
                             irr_w=np.ascontiguousarray(irr_w_pm, F32),
                             rjobs=rj, ijobs=ij))

    return dict(cat=cat, per_core=per_core, NRG=NRG, NIG=NIG, NSLOT=NSLOT,
                NPIX=cat.shape[0], A=A)


# ----------------------------------------------------------------------------
# device program
# ----------------------------------------------------------------------------

def _build_program(NPIX, NRG, NIG):
    f32 = mybir.dt.float32
    i32 = mybir.dt.int32
    MULT = mybir.AluOpType.mult
    ADD = mybir.AluOpType.add
    COPY = mybir.ActivationFunctionType.Copy
    GTOT = NRG + NIG
    OB = 4                       # output groups batched per DMA

    nc = bacc.Bacc("TRN2")
    cat_t = nc.dram_tensor("cat", [NPIX, C], f32, kind="ExternalInput")
    rix_t = nc.dram_tensor("reg_idx", [P, NRG * 2], i32, kind="ExternalInput")
    rw_t = nc.dram_tensor("reg_w", [P, NRG * 4], f32, kind="ExternalInput")
    iix_t = nc.dram_tensor("irr_idx", [P, NIG * 2], i32, kind="ExternalInput")
    iw_t = nc.dram_tensor("irr_w", [P, NIG * IRRW], f32, kind="ExternalInput")
    out_t = nc.dram_tensor("out", [P, GTOT * SIZE * C], f32,
                           kind="ExternalOutput")

    with TileContext(nc) as tc:
        with tc.tile_pool(name="const", bufs=1) as cpool, \
             tc.tile_pool(name="gat", bufs=10) as gpool, \
             tc.tile_pool(name="mid", bufs=6) as upool, \
             tc.tile_pool(name="fin", bufs=5) as opool, \
             tc.tile_pool(name="ob", bufs=3) as obpool:

            rix = cpool.tile([P, NRG * 2], i32)
            rw = cpool.tile([P, NRG * 4], f32)
            iix = cpool.tile([P, NIG * 2], i32)
            iw = cpool.tile([P, NIG * IRRW], f32)
            nc.sync.dma_start(rix[:, :], rix_t[:, :])
            nc.sync.dma_start(rw[:, :], rw_t[:, :])
            nc.sync.dma_start(iix[:, :], iix_t[:, :])
            nc.sync.dma_start(iw[:, :], iw_t[:, :])

            def y_stage(idx_ap_lo, idx_ap_hi, wl_ap, wh_ap):
                glo = gpool.tile([P, 9 * C], f32, tag="glo")
                ghi = gpool.tile([P, 9 * C], f32, tag="ghi")
                nc.gpsimd.indirect_dma_start(
                    out=glo[:, :], out_offset=None, in_=cat_t[:, :],
                    in_offset=bass.IndirectOffsetOnAxis(ap=idx_ap_lo, axis=0))
                nc.gpsimd.indirect_dma_start(
                    out=ghi[:, :], out_offset=None, in_=cat_t[:, :],
                    in_offset=bass.IndirectOffsetOnAxis(ap=idx_ap_hi, axis=0))
                m1 = upool.tile([P, 9 * C], f32, tag="m1")
                nc.scalar.activation(m1[:, :], glo[:, :], COPY, scale=wl_ap)
                u = upool.tile([P, 9 * C], f32, tag="u")
                nc.vector.scalar_tensor_tensor(
                    out=u[:, :], in0=ghi[:, :], scalar=wh_ap, in1=m1[:, :],
                    op0=MULT, op1=ADD)
                return u

            obuf = None
            ob_base = 0

            def flush(gi):
                nonlocal obuf, ob_base
                if obuf is not None:
                    nc.sync.dma_start(
                        out_t[:, ob_base * SIZE * C: gi * SIZE * C],
                        obuf[:, : (gi - ob_base) * SIZE * C])
                    obuf = None

            for gi in range(GTOT):
                if obuf is None:
                    obuf = obpool.tile([P, OB * SIZE * C], f32, tag="obuf")
                    ob_base = gi
                osl = obuf[:, (gi - ob_base) * SIZE * C:
                           (gi - ob_base + 1) * SIZE * C]
                if gi < NRG:
                    g = gi
                    u = y_stage(rix[:, 2 * g: 2 * g + 1], rix[:, 2 * g + 1: 2 * g + 2],
                                rw[:, 4 * g: 4 * g + 1], rw[:, 4 * g + 1: 4 * g + 2])
                    ur = u[:, :].rearrange("p (x c) -> p x c", c=C)
                    m2 = opool.tile([P, SIZE * C], f32, tag="m2")
                    m2r = m2[:, :].rearrange("p (x c) -> p x c", c=C)
                    nc.scalar.activation(m2r, ur[:, 0:8, :], COPY,
                                         scale=rw[:, 4 * g + 2: 4 * g + 3])
                    oslr = osl.rearrange("p (x c) -> p x c", c=C)
                    nc.vector.scalar_tensor_tensor(
                        out=oslr, in0=ur[:, 1:9, :],
                        scalar=rw[:, 4 * g + 3: 4 * g + 4], in1=m2r,
                        op0=MULT, op1=ADD)
                else:
                    g = gi - NRG
                    u = y_stage(iix[:, 2 * g: 2 * g + 1], iix[:, 2 * g + 1: 2 * g + 2],
                                iw[:, IRRW * g: IRRW * g + 1],
                                iw[:, IRRW * g + 1: IRRW * g + 2])
                    ur = u[:, :].rearrange("p (x c) -> p x c", c=C)
                    acc0 = opool.tile([P, SIZE * C], f32, tag="acc0")
                    acc1 = opool.tile([P, SIZE * C], f32, tag="acc1")
                    acc = [acc0, acc1]
                    tmp = opool.tile([P, SIZE * C], f32, tag="tmp")
                    wbase = IRRW * g + 4
                    for e in range(9):
                        u_e = ur[:, e:e + 1, :].to_broadcast([P, SIZE, C])
                        m_e = (iw[:, wbase + e * SIZE: wbase + (e + 1) * SIZE]
                               .rearrange("p (j u) -> p j u", u=1)
                               .to_broadcast([P, SIZE, C]))
                        dst = acc[0] if e == 0 else tmp
                        dstr = dst[:, :].rearrange("p (j c) -> p j c", c=C)
                        nc.vector.tensor_tensor(out=dstr, in0=u_e, in1=m_e, op=MULT)
                        if 0 < e < 8:
                            nc.vector.tensor_tensor(out=acc[e % 2][:, :],
                                                    in0=acc[(e - 1) % 2][:, :],
                                                    in1=tmp[:, :], op=ADD)
                        elif e == 8:
                            nc.vector.tensor_tensor(out=osl, in0=acc[1][:, :],
                                                    in1=tmp[:, :], op=ADD)
                if gi - ob_base + 1 == OB:
                    flush(gi + 1)
            flush(GTOT)

    nc.finalize()
    return nc


# ----------------------------------------------------------------------------
# entry point
# ----------------------------------------------------------------------------

def kernel(f0, f1, f2, pixel, batch_index):
    global LAST_RESULTS
    prep = _host_prep(f0, f1, f2, pixel, batch_index)
    NRG, NIG, NSLOT, A = prep["NRG"], prep["NIG"], prep["NSLOT"], prep["A"]

    nc = _build_program(prep["NPIX"], NRG, NIG)

    in_maps = []
    for ccc in range(N_CORES):
        pc = prep["per_core"][ccc]
        in_maps.append({"cat": prep["cat"], "reg_idx": pc["reg_idx"],
                        "reg_w": pc["reg_w"], "irr_idx": pc["irr_idx"],
                        "irr_w": pc["irr_w"]})

    res = run_bass_kernel_spmd(nc, in_maps, core_ids=list(range(N_CORES)),
                               trace=bool(os.environ.get("BASS_TRACE")))
    LAST_RESULTS = res

    out = np.zeros((A, 3, C, SIZE, SIZE), F32)
    irr0 = NRG * JOBS_PER_GROUP
    GTOT = NRG + NIG
    for ccc in range(N_CORES):
        pc = prep["per_core"][ccc]
        # device layout [128, GTOT*512] -> slot-major [GTOT*16, 8, 8, 64]
        raw = res.results[ccc]["out"].reshape(P, GTOT, SIZE * C)
        dev = (raw.transpose(1, 0, 2)
               .reshape(GTOT, JOBS_PER_GROUP, SIZE, SIZE, C)
               .reshape(-1, SIZE, SIZE, C))
        rj, ij = pc["rjobs"], pc["ijobs"]
        if len(rj):
            slabs = dev[:len(rj)].transpose(0, 3, 1, 2)
            out[rj[:, 1], rj[:, 0]] = slabs
        if len(ij):
            slabs = dev[irr0:irr0 + len(ij)].transpose(0, 3, 1, 2)
            out[ij[:, 1], ij[:, 0]] = slabs
    return out.reshape(A, 3 * C, SIZE, SIZE)

